# revision 14
# baseline (speedup 1.0000x reference)
"""Trainium2 Bass kernel for nn_ARGCNNet (2-layer gated relational GCN), v2.

Strategy (8 NeuronCores, graph/data parallel):
  - Nodes sharded by row: core c owns nodes [c*6250, (c+1)*6250).
  - Edges routed to the core owning their dst node, sorted by dst window,
    packed into 128-edge chunks (padding uniform across cores -> one SPMD
    program). Chunks split into A (permuted src < 32768) and B parts because
    dma_gather indices are int16.
  - Per-edge gates alpha1/alpha2 are pure functions of host-known inputs
    (edge_type/edge_distance + small tables) -> computed on HOST.
  - The alpha-scaled one-hot matrices (lhsT of the segment-sum matmuls) are
    HOST-precomputed in fp8e4 and streamed in, killing all on-device one-hot
    DVE work and the per-edge alpha gather.
  - Message path runs in fp8e4: xt = x@W1_msg cast to fp8, AllGather'ed in
    fp8 (half the bytes), per-edge rows gathered as 256B fp8 rows, and the
    segment-sum matmuls run fp8 x fp8 with DoubleRow perf mode (2 chunks per
    matmul, 2x PE rate). Root paths and dense GEMMs stay bf16.
  - AllGathers are chunked into 4 node-slabs and overlapped: AG(xt) slabs
    fire as dense1 finishes each slab; dense2 is interleaved into the edge-1
    loop so AG(ht) slabs fire while edge-1 still runs. Table row ids are
    permuted host-side to match the slab-concatenated AllGather layout.
  - h never touches DRAM: transposed on the PE into an SBUF slab for dense2.
  - Dropout masks are host-precomputed 0/1 fp8; the 1/(1-p) scale is folded
    into the ReLU activations.
"""

import os
import sys

import numpy as np

for _p in ("/opt/trn_rl_repo", "/root/.axon_site/_ro/trn_rl_repo"):
    if os.path.isdir(_p) and _p not in sys.path:
        sys.path.insert(0, _p)

import ml_dtypes

bf16 = ml_dtypes.bfloat16
f8 = ml_dtypes.float8_e4m3  # TRN FP8_EXP4 (matches for |x| <= 240)

N_NODES = 50000
N_EDGES = 800000
IN_DIM = 768
HID = 256
OUT = 9
OUTP = 16
N_TYPES = 50
N_DIST = 128
P_DROP = np.float32(0.4)
INV_KEEP = float(np.float32(1.0) / (np.float32(1.0) - P_DROP))

NCORES = 8
SHARD = N_NODES // NCORES  # 6250
P = 128
NW = (SHARD + P - 1) // P  # 49 windows per core
PADN = NW * P  # 6272
KT1 = IN_DIM // P  # 6
KT2 = HID // P  # 2
SPLIT = 32768  # int16 index limit for dma_gather
GW = 2  # windows per gather group
IDXCAP = 1024  # max indices per dma_gather call
DDS = 65536

# AllGather slabs (core-local row ranges). The slab boundary at local row
# 4096 puts the slab-0/1 table split exactly at 8*4096 = 32768 = SPLIT, so
# A-part gathers (int16 idx < 32768) depend only on slab 0's AllGather.
SLAB_STARTS = [0, 4096]
SLAB_LENS = [4096, 2154]
SLAB_WEND = [32, 49]  # dense window index (exclusive) per slab
NSLAB = 2


def _perm_ids():
    """Global node id -> permuted table row id (slab-concatenated AllGather
    layout: table = [slab0: core0..7 | slab1: core0..7 | ...])."""
    ids = np.arange(N_NODES, dtype=np.int64)
    c = ids // SHARD
    r = ids % SHARD
    s = np.minimum(r // 4096, 1)
    starts = np.asarray(SLAB_STARTS, dtype=np.int64)[s]
    lens = np.asarray(SLAB_LENS, dtype=np.int64)[s]
    return 8 * starts + c * lens + (r - starts)


def _wrap_idx(flat):
    """int16 flat index list -> [128, n/16] wrapped + replicated layout."""
    n = flat.size
    assert n % 16 == 0
    t = np.empty((P, n // 16), np.int16)
    for p in range(16):
        row = flat[p::16]
        for g in range(8):
            t[16 * g + p, :] = row
    return t


def _edge_alphas(et, ed, te, de, gw, gb):
    tg = te.astype(np.float64) @ gw[:100, 0].astype(np.float64)  # [50]
    dg = de.astype(np.float64) @ gw[100:, 0].astype(np.float64)  # [128]
    z = tg[et] + dg[ed] + float(gb)
    return (1.0 / (1.0 + np.exp(-z))).astype(np.float32)


def _prep_edges(edge_index, a1, a2):
    """Route/sort/pack edges; build per-core src16 + fp8 one-hot arrays.

    Edge windows are 64 dst nodes wide (NW64 = 98 per core): the one-hot
    lhsT blocks are [128 slots, 64 dsts], halving the OH bytes and the
    layer-2 matmul columns. A gather group = 4 consecutive 64-windows
    (256 dsts)."""
    src = np.asarray(edge_index[0]).astype(np.int64)
    dst = np.asarray(edge_index[1]).astype(np.int64)
    perm = _perm_ids()
    psrc = perm[src]
    owner = dst // SHARD

    NW64 = 2 * NW  # 98
    per_core = []
    cntA = np.zeros((NCORES, NW64), np.int64)
    cntB = np.zeros((NCORES, NW64), np.int64)
    for c in range(NCORES):
        m = owner == c
        dstl = dst[m] - c * SHARD
        ps_ = psrc[m]
        isB = (ps_ >= SPLIT).astype(np.int64)
        wid = dstl >> 6
        key = wid * 2 + isB
        order = np.argsort(key, kind="stable")
        per_core.append(
            (dstl[order], ps_[order], a1[m][order], a2[m][order], isB[order])
        )
        cntA[c] = np.bincount(wid[isB == 0], minlength=NW64)
        cntB[c] = np.bincount(wid[isB == 1], minlength=NW64)

    cwA = np.maximum(1, (cntA.max(axis=0) + P - 1) // P)  # [NW64]
    cwB = np.maximum(1, (cntB.max(axis=0) + P - 1) // P)

    # group = 4 consecutive 64-windows (2 superwindows)
    groups = [list(range(g, min(g + 4, NW64))) for g in range(0, NW64, 4)]
    colA = {}
    colB = {}
    callsA = []  # (col0, ncols) per group
    callsB = []
    cur = 0
    for ws in groups:
        c0 = cur
        for w in ws:
            colA[w] = cur
            cur += int(cwA[w])
        callsA.append((c0, cur - c0))
        c0 = cur
        for w in ws:
            colB[w] = cur
            cur += int(cwB[w])
        callsB.append((c0, cur - c0))
    C = cur

    meta = {
        "cwA": cwA,
        "cwB": cwB,
        "colA": colA,
        "colB": colB,
        "callsA": callsA,
        "callsB": callsB,
        "groups": groups,
        "C": C,
    }

    colA_arr = np.array([colA[w] for w in range(NW64)])
    colB_arr = np.array([colB[w] for w in range(NW64)])
    per_core_arrays = []
    for c in range(NCORES):
        dstl, ps_, a1c, a2c, isB = per_core[c]
        wid = dstl >> 6
        keys = wid * 2 + isB
        cnt = np.bincount(keys, minlength=2 * NW64)
        start = np.concatenate([[0], np.cumsum(cnt)[:-1]])
        rank = np.arange(dstl.size) - start[keys]
        colbase = np.where(isB == 0, colA_arr[wid], colB_arr[wid])
        slot = (colbase + (rank >> 7)) * P + (rank & 127)

        srcrel = np.zeros(C * P, np.int16)
        srcrel[slot] = np.where(isB == 1, ps_ - SPLIT, ps_).astype(np.int16)

        # one-hot (alpha-scaled) lhsT arrays: [slot_p, col, dst_low64]
        flat = (slot & 127) * (C * 64) + (slot >> 7) * 64 + (dstl & 63)
        oh = np.zeros(P * C * 64, np.float32)
        oh[flat] = a1c
        OH1 = oh.reshape(P, C * 64).astype(f8)
        oh[flat] = a2c
        OH2 = oh.reshape(P, C * 64).astype(f8)
        per_core_arrays.append((_wrap_idx(srcrel), OH1, OH2))
    return meta, per_core_arrays


def _build_program(meta, sim_mode=False):
    import concourse.bacc as bacc
    import concourse.bass as bass  # noqa: F401
    import concourse.mybir as mybir
    import concourse.tile as tile

    A = mybir.AluOpType
    F = mybir.ActivationFunctionType
    dt = mybir.dt
    DR = mybir.MatmulPerfMode.DoubleRow

    C = meta["C"]
    cwA, cwB = meta["cwA"], meta["cwB"]
    colA, colB = meta["colA"], meta["colB"]
    callsA, callsB = meta["callsA"], meta["callsB"]
    groups = meta["groups"]

    nc = bacc.Bacc(
        "TRN2", target_bir_lowering=False, debug=False,
        num_devices=(1 if sim_mode else NCORES),
        dynamic_dma_scratch_size=DDS,
        num_swdge_queues=4,
    )

    def inp(name, shape, d):
        return nc.dram_tensor(name, shape, d, kind="ExternalInput")

    xT = inp("xT", [IN_DIM, PADN], dt.bfloat16)
    W1 = inp("W1", [IN_DIM, 2 * HID], dt.bfloat16)  # [msg | root]
    W2 = inp("W2", [HID, 2 * OUTP], dt.bfloat16)  # [msg | root] padded
    b1row = inp("b1row", [1, HID], dt.bfloat16)
    b2c = inp("b2c", [1, OUTP], dt.bfloat16)
    ones_bf = inp("ones_bf", [1, P], dt.bfloat16)
    ident_in = inp("ident", [P, P], dt.bfloat16)
    m1_in = inp("m1", [PADN, HID], dt.float8e4)
    m2T_in = inp("m2T", [OUTP, PADN], dt.float8e4)
    src16_in = inp("src16", [P, C * 8], dt.int16)
    OH1_in = inp("OH1", [P, C * 64], dt.float8e4)
    OH2_in = inp("OH2", [P, C * 64], dt.float8e4)

    yT = nc.dram_tensor("yT", [OUTP, PADN], dt.float32, kind="ExternalOutput")

    xt_loc = nc.dram_tensor("xt_loc", [PADN, HID], dt.float8e4, kind="Internal")
    xt_full = nc.dram_tensor(
        "xt_full", [N_NODES, HID], dt.float8e4, kind="Internal",
        addr_space="Shared",
    )
    ht_loc = nc.dram_tensor("ht_loc", [PADN, P], dt.bfloat16, kind="Internal")
    ht_full = nc.dram_tensor(
        "ht_full", [N_NODES, P], dt.bfloat16, kind="Internal",
        addr_space="Shared",
    )

    rg = [list(range(NCORES))]
    _qrr = [0]

    def dg_raw(out_ap, in_ap, idxs_ap, num_idxs, elem_size, stride_256,
               queue=None):
        eng = nc.gpsimd
        if queue is None:
            q = _qrr[0]
            _qrr[0] = (q + 1) % 3
        else:
            q = queue
        _in_ap = eng.lower_ap_dma(in_ap, for_custom_bir_dma=True)
        _idxs_ap = eng.lower_ap(idxs_ap)
        _out_ap = eng.lower_ap(out_ap)
        return eng.add_instruction(
            mybir.InstDMAGatherAnt(
                name=nc.get_next_instruction_name(),
                ins=[*_in_ap, _idxs_ap, eng.lower_val_access(eng.to_reg(num_idxs))],
                outs=[_out_ap],
                transpose=False,
                num_idxs=num_idxs,
                elem_size=elem_size,
                stride_bytes_256=stride_256,
                gen_mode=0,
                single_packet=True,
                queue_num=q,
                sbuf_tokens_per_rank=0,
                sbuf_free_dim_per_rank=0,
                sbuf_free_dim_pad_per_rank=0,
                sbuf_byte_offset=0,
            )
        )

    def allgather(src_dram, dst_dram, s):
        a, ln = SLAB_STARTS[s], SLAB_LENS[s]
        if sim_mode:
            for cc in range(NCORES):
                nc.sync.dma_start(
                    dst_dram[8 * a + cc * ln : 8 * a + (cc + 1) * ln, :],
                    src_dram[a : a + ln, :],
                )
        else:
            nc.gpsimd.collective_compute(
                "AllGather",
                A.bypass,
                replica_groups=rg,
                ins=[src_dram[a : a + ln, :]],
                outs=[dst_dram[8 * a : 8 * (a + ln), :]],
            )

    maxGA = max(n for _, n in callsA)
    maxGB = max(n for _, n in callsB)
    maxG = max(
        int(sum(cwA[w] + cwB[w] for w in ws)) for ws in groups
    )


    with tile.TileContext(nc) as tc:
        import contextlib

        ctx = contextlib.ExitStack()
        sb = ctx.enter_context(tc.tile_pool(name="sb", bufs=1))
        sb3 = ctx.enter_context(tc.tile_pool(name="sb3", bufs=3))
        psp = ctx.enter_context(tc.tile_pool(name="psp", bufs=1, space="PSUM"))

        # ---------- resident loads ----------
        src16_sb = sb.tile([P, C * 8], dt.int16)
        nc.sync.dma_start(src16_sb[:], src16_in[:])
        ones_bf_s = sb.tile([1, P], dt.bfloat16)
        nc.sync.dma_start(ones_bf_s[:], ones_bf[:])
        b1row_s = sb.tile([1, HID], dt.bfloat16)
        nc.sync.dma_start(b1row_s[:], b1row[:])
        b2c_s = sb.tile([1, OUTP], dt.bfloat16)
        nc.sync.dma_start(b2c_s[:], b2c[:])
        ident_s = sb.tile([P, P], dt.bfloat16)
        nc.sync.dma_start(ident_s[:], ident_in[:])
        m2T_s = sb.tile([OUTP, PADN], dt.float8e4)
        nc.sync.dma_start(m2T_s[:], m2T_in[:])
        m1_slab = sb.tile([64, 2 * NW, HID], dt.float8e4)
        nc.sync.dma_start(
            m1_slab[:], m1_in[0:PADN, :].rearrange("(w p) h -> p w h", p=64)
        )

        W1_s = []
        for k in range(KT1):
            t = sb.tile([P, 2 * HID], dt.bfloat16, name=f"W1_s{k}")
            nc.sync.dma_start(t[:], W1[k * P : (k + 1) * P, :])
            W1_s.append(t)
        W2_s = []
        for k in range(KT2):
            t = sb.tile([P, 2 * OUTP], dt.bfloat16, name=f"W2_s{k}")
            nc.sync.dma_start(t[:], W2[k * P : (k + 1) * P, :])
            W2_s.append(t)

        root1_slab = sb.tile([P, NW * HID], dt.bfloat16)
        root2T_slab = sb.tile([OUTP, PADN], dt.bfloat16)
        hT_slab = []
        for k in range(KT2):
            t = sb.tile([P, PADN], dt.bfloat16, name=f"hT_slab{k}")
            hT_slab.append(t)

        # pre-zeroed fp8 pad buffers for the ht table rows
        htpad = []
        for i in range(2):
            t = sb.tile([P, P], dt.bfloat16, name=f"htpad{i}")
            nc.vector.memset(t[:], 0.0)
            htpad.append(t)

        # ---------- dense1 + chunked AllGather(xt) ----------
        slab_idx = 0
        for m in range(NW):
            ps = psp.tile([P, 2 * HID], dt.float32, space="PSUM", tag="d1", bufs=2)
            xt_k = sb3.tile([P, KT1, P], dt.bfloat16, tag="xTt", bufs=3)
            nc.sync.dma_start(
                xt_k[:],
                xT[:, m * P : (m + 1) * P].rearrange("(k p) n -> p k n", k=KT1),
            )
            for k in range(KT1):
                nc.tensor.matmul(
                    ps[:], lhsT=xt_k[:, k, :], rhs=W1_s[k][:],
                    start=(k == 0), stop=False,
                )
            nc.tensor.matmul(
                ps[:, HID : 2 * HID],
                lhsT=ones_bf_s[:], rhs=b1row_s[:],
                start=False, stop=True,
            )
            xt_t = sb3.tile([P, HID], dt.float8e4, tag="xt_t")
            nc.scalar.copy(xt_t[:], ps[:, 0:HID])
            nc.sync.dma_start(xt_loc[m * P : (m + 1) * P, :], xt_t[:])
            nc.vector.tensor_copy(
                out=root1_slab[:, m * HID : (m + 1) * HID],
                in_=ps[:, HID : 2 * HID],
            )
            if m + 1 == SLAB_WEND[slab_idx]:
                allgather(xt_loc, xt_full, slab_idx)
                slab_idx += 1

        # ---- edge layer 1 (+ interleaved dense2 + AG(ht)) --------------
        # groups = 4 edge-windows (64 dsts each) = 2 superwindows of 128
        cap = IDXCAP // P
        slab_idx = 0
        for gi, ws in enumerate(groups):
            c0A, nA = callsA[gi]
            c0B, nB = callsB[gi]
            rowsA = sb3.tile([P, maxGA, HID], dt.float8e4, tag="rows1A", bufs=2)
            for o in range(0, nA, cap):
                n_ = min(cap, nA - o)
                dg_raw(
                    rowsA[:, o : o + n_, :], xt_full[0:SPLIT, :],
                    src16_sb[:, (c0A + o) * 8 : (c0A + o + n_) * 8],
                    n_ * P, HID, 1,
                )
            rowsB = sb3.tile([P, maxGB, HID], dt.float8e4, tag="rows1B", bufs=2)
            for o in range(0, nB, cap):
                n_ = min(cap, nB - o)
                dg_raw(
                    rowsB[:, o : o + n_, :], xt_full[SPLIT:, :],
                    src16_sb[:, (c0B + o) * 8 : (c0B + o + n_) * 8],
                    n_ * P, HID, 1, queue=3,
                )
            oh1_t = sb3.tile([P, maxG, 64], dt.float8e4, tag="oh1", bufs=2)
            gc0 = c0A
            gcols = nA + nB
            nc.sync.dma_start(
                oh1_t[:, 0:gcols, :], OH1_in[:, gc0 * 64 : (gc0 + gcols) * 64]
            )

            for w64 in ws:
                sw, half = w64 >> 1, w64 & 1
                acols = [
                    (rowsA, colA[w64] - c0A, colA[w64] - gc0, int(cwA[w64]))
                ]
                bcols = [
                    (rowsB, colB[w64] - c0B, colB[w64] - gc0, int(cwB[w64]))
                ]
                ps_b = psp.tile(
                    [64, HID], dt.float32, space="PSUM", tag="big", bufs=2
                )
                first = True
                for rt, rc0, oc0, ncol in acols + bcols:
                    j = 0
                    while j + 2 <= ncol:
                        nc.tensor.matmul(
                            ps_b[:],
                            lhsT=oh1_t[:, oc0 + j : oc0 + j + 2, :],
                            rhs=rt[:, rc0 + j : rc0 + j + 2, :],
                            start=first, stop=False, perf_mode=DR,
                        )
                        first = False
                        j += 2
                    if j < ncol:
                        nc.tensor.matmul(
                            ps_b[:],
                            lhsT=oh1_t[:, oc0 + j, :],
                            rhs=rt[:, rc0 + j, :],
                            start=first, stop=False,
                        )
                        first = False
                # + root1 rows [half*64, half*64+64) (includes b1); the
                # identity-column slice also shifts partitions 64:128 -> 0:64
                nc.tensor.matmul(
                    ps_b[:],
                    lhsT=ident_s[:, half * 64 : (half + 1) * 64],
                    rhs=root1_slab[:, sw * HID : (sw + 1) * HID],
                    start=False, stop=True,
                )
                t0 = sb3.tile([64, HID], dt.bfloat16, tag="t0", bufs=2)
                nc.vector.tensor_tensor(
                    out=t0[:], in0=ps_b[:], in1=m1_slab[:, w64, :], op=A.mult
                )
                h64 = sb3.tile([64, HID], dt.bfloat16, tag="h64", bufs=2)
                nc.scalar.activation(h64[:], t0[:], F.Relu, scale=INV_KEEP)

                # transpose into the hT slab ([64,128] -> [128,64])
                tp = psp.tile(
                    [P, 2 * 64], dt.bfloat16, space="PSUM", tag="tp", bufs=1
                )
                for k in range(KT2):
                    nc.tensor.transpose(
                        out=tp[:, k * 64 : (k + 1) * 64],
                        in_=h64[:, k * P : (k + 1) * P],
                        identity=ident_s[0:64, 0:64],
                    )
                    nc.scalar.copy(
                        hT_slab[k][:, w64 * 64 : (w64 + 1) * 64],
                        tp[:, k * 64 : (k + 1) * 64],
                    )

                if half == 1:
                    # dense2 for completed superwindow sw
                    psm = psp.tile(
                        [P, OUTP], dt.float32, space="PSUM", tag="pm", bufs=1
                    )
                    for k in range(KT2):
                        nc.tensor.matmul(
                            psm[:],
                            lhsT=hT_slab[k][:, sw * P : (sw + 1) * P],
                            rhs=W2_s[k][:, 0:OUTP],
                            start=(k == 0), stop=(k == KT2 - 1),
                        )
                    hp = htpad[sw % 2]
                    nc.scalar.copy(hp[:, 0:OUTP], psm[:])
                    nc.sync.dma_start(ht_loc[sw * P : (sw + 1) * P, :], hp[:])

                    psr = psp.tile(
                        [OUTP, P], dt.float32, space="PSUM", tag="pg", bufs=2
                    )
                    for k in range(KT2):
                        nc.tensor.matmul(
                            psr[:],
                            lhsT=W2_s[k][:, OUTP : 2 * OUTP],
                            rhs=hT_slab[k][:, sw * P : (sw + 1) * P],
                            start=(k == 0), stop=False,
                        )
                    nc.tensor.matmul(
                        psr[:], lhsT=b2c_s[:], rhs=ones_bf_s[:],
                        start=False, stop=True,
                    )
                    nc.scalar.copy(root2T_slab[:, sw * P : (sw + 1) * P], psr[:])

                    if sw + 1 == SLAB_WEND[slab_idx]:
                        allgather(ht_loc, ht_full, slab_idx)
                        slab_idx += 1

        # ---- edge layer 2 ----------------------------------------------
        for gi, ws in enumerate(groups):
            c0A, nA = callsA[gi]
            c0B, nB = callsB[gi]
            rows2A = sb3.tile([P, maxGA, OUTP], dt.bfloat16, tag="rows2A", bufs=2)
            for o in range(0, nA, cap):
                n_ = min(cap, nA - o)
                dg_raw(
                    rows2A[:, o : o + n_, :], ht_full[0:SPLIT, 0:OUTP],
                    src16_sb[:, (c0A + o) * 8 : (c0A + o + n_) * 8],
                    n_ * P, OUTP, 1,
                )
            rows2B = sb3.tile([P, maxGB, OUTP], dt.bfloat16, tag="rows2B", bufs=2)
            for o in range(0, nB, cap):
                n_ = min(cap, nB - o)
                dg_raw(
                    rows2B[:, o : o + n_, :], ht_full[SPLIT:, 0:OUTP],
                    src16_sb[:, (c0B + o) * 8 : (c0B + o + n_) * 8],
                    n_ * P, OUTP, 1, queue=3,
                )
            oh2_t = sb3.tile([P, maxG, 64], dt.float8e4, tag="oh2", bufs=2)
            gc0 = c0A
            gcols = nA + nB
            nc.sync.dma_start(
                oh2_t[:, 0:gcols, :], OH2_in[:, gc0 * 64 : (gc0 + gcols) * 64]
            )

            for w64 in ws:
                acols = [
                    (rows2A, colA[w64] - c0A, colA[w64] - gc0, int(cwA[w64]))
                ]
                bcols = [
                    (rows2B, colB[w64] - c0B, colB[w64] - gc0, int(cwB[w64]))
                ]
                psg_full = psp.tile(
                    [OUTP, P], dt.float32, space="PSUM", tag="pg", bufs=2
                )
                psg = psg_full[:, 0:64]
                first = True
                for rt, rc0, oc0, ncol in acols + bcols:
                    for j in range(ncol):
                        nc.tensor.matmul(
                            psg[:],
                            lhsT=rt[:, rc0 + j, :],
                            rhs=oh2_t[:, oc0 + j, :],
                            start=first, stop=False,
                        )
                        first = False
                # + root2 (includes b2)
                nc.tensor.matmul(
                    psg[:],
                    lhsT=ident_s[0:OUTP, 0:OUTP],
                    rhs=root2T_slab[:, w64 * 64 : (w64 + 1) * 64],
                    start=False, stop=True,
                )
                t2 = sb3.tile([OUTP, 64], dt.float32, tag="t2", bufs=2)
                nc.vector.tensor_tensor(
                    out=t2[:], in0=psg[:],
                    in1=m2T_s[:, w64 * 64 : (w64 + 1) * 64], op=A.mult,
                )
                yt_t = sb3.tile([OUTP, 64], dt.float32, tag="yt_t", bufs=2)
                nc.scalar.activation(yt_t[:], t2[:], F.Relu, scale=INV_KEEP)
                nc.sync.dma_start(yT[:, w64 * 64 : (w64 + 1) * 64], yt_t[:])
        ctx.close()

    nc.compile()
    return nc


def _build_noop_program(meta=None):
    """Same I/O signature as the real program, near-empty body — used to
    measure PJRT dispatch overhead for wall-clock benchmarking."""
    import concourse.bacc as bacc
    import concourse.mybir as mybir
    import concourse.tile as tile

    dt = mybir.dt
    C = meta["C"] if meta else 848
    nc = bacc.Bacc(
        "TRN2", target_bir_lowering=False, debug=False, num_devices=NCORES,
        dynamic_dma_scratch_size=DDS, num_swdge_queues=4,
    )

    def inp(name, shape, d):
        return nc.dram_tensor(name, shape, d, kind="ExternalInput")

    inp("xT", [IN_DIM, PADN], dt.bfloat16)
    inp("W1", [IN_DIM, 2 * HID], dt.bfloat16)
    inp("W2", [HID, 2 * OUTP], dt.bfloat16)
    inp("b1row", [1, HID], dt.bfloat16)
    inp("b2c", [1, OUTP], dt.bfloat16)
    inp("ones_bf", [1, P], dt.bfloat16)
    inp("ident", [P, P], dt.bfloat16)
    m1 = inp("m1", [PADN, HID], dt.float8e4)
    inp("m2T", [OUTP, PADN], dt.float8e4)
    inp("src16", [P, C * 8], dt.int16)
    inp("OH1", [P, C * 64], dt.float8e4)
    inp("OH2", [P, C * 64], dt.float8e4)
    yT = nc.dram_tensor("yT", [OUTP, PADN], dt.float32, kind="ExternalOutput")
    with tile.TileContext(nc) as tc:
        with tc.tile_pool(name="sb", bufs=1) as sb:
            t = sb.tile([OUTP, P], dt.float8e4)
            nc.sync.dma_start(t[:], m1[0:OUTP, 0:P])
            t2 = sb.tile([OUTP, P], dt.float32)
            nc.vector.tensor_copy(out=t2[:], in_=t[:])
            nc.sync.dma_start(yT[:, 0:P], t2[:])
    nc.compile()
    return nc


def _stage_inputs(inputs, per_core_arrays):
    x = np.asarray(inputs["x"], np.float32)
    W1m = np.asarray(inputs["W1_msg"], np.float32)
    W1r = np.asarray(inputs["W1_root"], np.float32)
    b1 = np.asarray(inputs["b1"], np.float32)
    W2m = np.asarray(inputs["W2_msg"], np.float32)
    W2r = np.asarray(inputs["W2_root"], np.float32)
    b2 = np.asarray(inputs["b2"], np.float32)
    drop1 = np.asarray(inputs["drop1"], np.float32)
    drop2 = np.asarray(inputs["drop2"], np.float32)

    W1cat = np.concatenate([W1m, W1r], axis=1).astype(bf16)  # [768,512]
    W2cat = np.zeros((HID, 2 * OUTP), np.float32)
    W2cat[:, 0:OUT] = W2m
    W2cat[:, OUTP : OUTP + OUT] = W2r
    W2cat = W2cat.astype(bf16)
    b1row = b1.reshape(1, HID).astype(bf16)
    b2c = np.zeros((1, OUTP), np.float32)
    b2c[0, :OUT] = b2
    b2c = b2c.astype(bf16)
    ones_bf_a = np.ones((1, P), bf16)
    ident_a = np.eye(P, dtype=np.float32).astype(bf16)

    common = {
        "W1": W1cat,
        "W2": W2cat,
        "b1row": b1row,
        "b2c": b2c,
        "ones_bf": ones_bf_a,
        "ident": ident_a,
    }

    in_maps = []
    for c in range(NCORES):
        lo, hi = c * SHARD, (c + 1) * SHARD
        xTp = np.ones((IN_DIM, PADN), np.float32)
        xTp[:, :SHARD] = x[lo:hi].T
        m1p = np.ones((PADN, HID), np.float32)
        m1p[:SHARD] = drop1[lo:hi] >= P_DROP
        m2Tp = np.zeros((OUTP, PADN), np.float32)
        m2Tp[:OUT, :SHARD] = (drop2[lo:hi] >= P_DROP).T
        src16, OH1, OH2 = per_core_arrays[c]
        in_maps.append(
            {
                **common,
                "xT": xTp.astype(bf16),
                "m1": m1p.astype(f8),
                "m2T": m2Tp.astype(f8),
                "src16": src16,
                "OH1": OH1,
                "OH2": OH2,
            }
        )
    return in_maps


def _run(inputs, trace=False, trace_kwargs=None):
    from concourse import bass_utils

    et = np.asarray(inputs["edge_type"]).astype(np.int64)
    ed = np.asarray(inputs["edge_distance"]).astype(np.int64)
    a1 = _edge_alphas(
        et, ed, np.asarray(inputs["te1"], np.float32),
        np.asarray(inputs["de1"], np.float32),
        np.asarray(inputs["g1_w"], np.float32),
        np.asarray(inputs["g1_b"]).reshape(-1)[0],
    )
    a2 = _edge_alphas(
        et, ed, np.asarray(inputs["te2"], np.float32),
        np.asarray(inputs["de2"], np.float32),
        np.asarray(inputs["g2_w"], np.float32),
        np.asarray(inputs["g2_b"]).reshape(-1)[0],
    )
    meta, per_core_arrays = _prep_edges(inputs["edge_index"], a1, a2)
    nc = _build_program(meta)
    in_maps = _stage_inputs(inputs, per_core_arrays)
    res = bass_utils.run_bass_kernel_spmd(
        nc,
        in_maps,
        core_ids=list(range(NCORES)),
        trace=trace,
        **(trace_kwargs or {}),
    )
    parts = []
    for c in range(NCORES):
        yTa = res.results[c]["yT"]
        parts.append(np.ascontiguousarray(yTa[:OUT, :SHARD].T))
    y = np.concatenate(parts, axis=0).astype(np.float32)
    return y, res


def kernel(**inputs) -> np.ndarray:
    y, _ = _run(inputs, trace=False)
    return y


# revision 15
# speedup vs baseline: 1.2056x; 1.2056x over previous
"""Trainium2 Bass kernel for nn_ARGCNNet (2-layer gated relational GCN), v2.

Strategy (8 NeuronCores, graph/data parallel):
  - Nodes sharded by row: core c owns nodes [c*6250, (c+1)*6250).
  - Edges routed to the core owning their dst node, sorted by dst window,
    packed into 128-edge chunks (padding uniform across cores -> one SPMD
    program). Chunks split into A (permuted src < 32768) and B parts because
    dma_gather indices are int16.
  - Per-edge gates alpha1/alpha2 are pure functions of host-known inputs
    (edge_type/edge_distance + small tables) -> computed on HOST.
  - The alpha-scaled one-hot matrices (lhsT of the segment-sum matmuls) are
    HOST-precomputed in fp8e4 and streamed in, killing all on-device one-hot
    DVE work and the per-edge alpha gather.
  - Message path runs in fp8e4: xt = x@W1_msg cast to fp8, AllGather'ed in
    fp8 (half the bytes), per-edge rows gathered as 256B fp8 rows, and the
    segment-sum matmuls run fp8 x fp8 with DoubleRow perf mode (2 chunks per
    matmul, 2x PE rate). Root paths and dense GEMMs stay bf16.
  - AllGathers are chunked into 4 node-slabs and overlapped: AG(xt) slabs
    fire as dense1 finishes each slab; dense2 is interleaved into the edge-1
    loop so AG(ht) slabs fire while edge-1 still runs. Table row ids are
    permuted host-side to match the slab-concatenated AllGather layout.
  - h never touches DRAM: transposed on the PE into an SBUF slab for dense2.
  - Dropout masks are host-precomputed 0/1 fp8; the 1/(1-p) scale is folded
    into the ReLU activations.
"""

import os
import sys

import numpy as np

for _p in ("/opt/trn_rl_repo", "/root/.axon_site/_ro/trn_rl_repo"):
    if os.path.isdir(_p) and _p not in sys.path:
        sys.path.insert(0, _p)

import ml_dtypes

bf16 = ml_dtypes.bfloat16
f8 = ml_dtypes.float8_e4m3  # TRN FP8_EXP4 (matches for |x| <= 240)

N_NODES = 50000
N_EDGES = 800000
IN_DIM = 768
HID = 256
OUT = 9
OUTP = 16
N_TYPES = 50
N_DIST = 128
P_DROP = np.float32(0.4)
INV_KEEP = float(np.float32(1.0) / (np.float32(1.0) - P_DROP))

NCORES = 8
SHARD = N_NODES // NCORES  # 6250
P = 128
NW = (SHARD + P - 1) // P  # 49 windows per core
PADN = NW * P  # 6272
KT1 = IN_DIM // P  # 6
KT2 = HID // P  # 2
SPLIT = 32768  # int16 index limit for dma_gather
GW = 2  # windows per gather group
IDXCAP = 1024  # max indices per dma_gather call
DDS = 65536

# AllGather slabs (core-local row ranges). The slab boundary at local row
# 4096 puts the slab-0/1 table split exactly at 8*4096 = 32768 = SPLIT, so
# A-part gathers (int16 idx < 32768) depend only on slab 0's AllGather.
SLAB_STARTS = [0, 4096]
SLAB_LENS = [4096, 2154]
SLAB_WEND = [32, 49]  # dense window index (exclusive) per slab
NSLAB = 2


def _perm_ids():
    """Global node id -> permuted table row id (slab-concatenated AllGather
    layout: table = [slab0: core0..7 | slab1: core0..7 | ...])."""
    ids = np.arange(N_NODES, dtype=np.int64)
    c = ids // SHARD
    r = ids % SHARD
    s = np.minimum(r // 4096, 1)
    starts = np.asarray(SLAB_STARTS, dtype=np.int64)[s]
    lens = np.asarray(SLAB_LENS, dtype=np.int64)[s]
    return 8 * starts + c * lens + (r - starts)


def _wrap_idx(flat):
    """int16 flat index list -> [128, n/16] wrapped + replicated layout."""
    n = flat.size
    assert n % 16 == 0
    t = np.empty((P, n // 16), np.int16)
    for p in range(16):
        row = flat[p::16]
        for g in range(8):
            t[16 * g + p, :] = row
    return t


def _edge_alphas(et, ed, te, de, gw, gb):
    tg = te.astype(np.float64) @ gw[:100, 0].astype(np.float64)  # [50]
    dg = de.astype(np.float64) @ gw[100:, 0].astype(np.float64)  # [128]
    z = tg[et] + dg[ed] + float(gb)
    return (1.0 / (1.0 + np.exp(-z))).astype(np.float32)


def _prep_edges(edge_index, a1, a2):
    """Route/sort/pack edges; build per-core src16 + fp8 one-hot arrays."""
    src = np.asarray(edge_index[0]).astype(np.int64)
    dst = np.asarray(edge_index[1]).astype(np.int64)
    perm = _perm_ids()
    psrc = perm[src]
    owner = dst // SHARD

    per_core = []
    cntA = np.zeros((NCORES, NW), np.int64)
    cntB = np.zeros((NCORES, NW), np.int64)
    for c in range(NCORES):
        m = owner == c
        dstl = dst[m] - c * SHARD
        ps_ = psrc[m]
        isB = (ps_ >= SPLIT).astype(np.int64)
        wid = dstl >> 7
        key = wid * 2 + isB
        order = np.argsort(key, kind="stable")
        per_core.append(
            (dstl[order], ps_[order], a1[m][order], a2[m][order], isB[order])
        )
        cntA[c] = np.bincount(wid[isB == 0], minlength=NW)
        cntB[c] = np.bincount(wid[isB == 1], minlength=NW)

    cwA = np.maximum(1, (cntA.max(axis=0) + P - 1) // P)  # [NW]
    cwB = np.maximum(1, (cntB.max(axis=0) + P - 1) // P)

    groups = [list(range(g, min(g + GW, NW))) for g in range(0, NW, GW)]
    colA = {}
    colB = {}
    callsA = []  # (col0, ncols) per group
    callsB = []
    cur = 0
    for ws in groups:
        c0 = cur
        for w in ws:
            colA[w] = cur
            cur += int(cwA[w])
        callsA.append((c0, cur - c0))
        c0 = cur
        for w in ws:
            colB[w] = cur
            cur += int(cwB[w])
        callsB.append((c0, cur - c0))
    C = cur

    meta = {
        "cwA": cwA,
        "cwB": cwB,
        "colA": colA,
        "colB": colB,
        "callsA": callsA,
        "callsB": callsB,
        "groups": groups,
        "C": C,
    }

    colA_arr = np.array([colA[w] for w in range(NW)])
    colB_arr = np.array([colB[w] for w in range(NW)])
    per_core_arrays = []
    for c in range(NCORES):
        dstl, ps_, a1c, a2c, isB = per_core[c]
        wid = dstl >> 7
        keys = wid * 2 + isB
        cnt = np.bincount(keys, minlength=2 * NW)
        start = np.concatenate([[0], np.cumsum(cnt)[:-1]])
        rank = np.arange(dstl.size) - start[keys]
        colbase = np.where(isB == 0, colA_arr[wid], colB_arr[wid])
        slot = (colbase + (rank >> 7)) * P + (rank & 127)

        srcrel = np.zeros(C * P, np.int16)
        srcrel[slot] = np.where(isB == 1, ps_ - SPLIT, ps_).astype(np.int16)

        # one-hot (alpha-scaled) lhsT arrays: [slot_p, col, dst_low]
        flat = (slot & 127) * (C * P) + (slot >> 7) * P + (dstl & 127)
        oh = np.zeros(P * C * P, np.float32)
        oh[flat] = a1c
        OH1 = oh.reshape(P, C * P).astype(f8)
        oh[flat] = a2c
        OH2 = oh.reshape(P, C * P).astype(f8)
        per_core_arrays.append((_wrap_idx(srcrel), OH1, OH2))
    return meta, per_core_arrays


def _build_program(meta, sim_mode=False):
    import concourse.bacc as bacc
    import concourse.bass as bass  # noqa: F401
    import concourse.mybir as mybir
    import concourse.tile as tile

    A = mybir.AluOpType
    F = mybir.ActivationFunctionType
    dt = mybir.dt
    DR = mybir.MatmulPerfMode.DoubleRow

    C = meta["C"]
    cwA, cwB = meta["cwA"], meta["cwB"]
    colA, colB = meta["colA"], meta["colB"]
    callsA, callsB = meta["callsA"], meta["callsB"]
    groups = meta["groups"]

    nc = bacc.Bacc(
        "TRN2", target_bir_lowering=False, debug=False,
        num_devices=(1 if sim_mode else NCORES),
        dynamic_dma_scratch_size=DDS,
        num_swdge_queues=4,
    )

    def inp(name, shape, d):
        return nc.dram_tensor(name, shape, d, kind="ExternalInput")

    xT = inp("xT", [IN_DIM, PADN], dt.bfloat16)
    W1 = inp("W1", [IN_DIM, 2 * HID], dt.bfloat16)  # [msg | root]
    W2 = inp("W2", [HID, 2 * OUTP], dt.bfloat16)  # [msg | root] padded
    b1row = inp("b1row", [1, HID], dt.bfloat16)
    b2c = inp("b2c", [1, OUTP], dt.bfloat16)
    ones_bf = inp("ones_bf", [1, P], dt.bfloat16)
    ident_in = inp("ident", [P, P], dt.bfloat16)
    m1_in = inp("m1", [PADN, HID], dt.float8e4)
    m2T_in = inp("m2T", [OUTP, PADN], dt.float8e4)
    src16_in = inp("src16", [P, C * 8], dt.int16)
    OH1_in = inp("OH1", [P, C * P], dt.float8e4)
    OH2_in = inp("OH2", [P, C * P], dt.float8e4)

    yT = nc.dram_tensor("yT", [OUTP, PADN], dt.float32, kind="ExternalOutput")

    xt_loc = nc.dram_tensor("xt_loc", [PADN, HID], dt.float8e4, kind="Internal")
    xt_full = nc.dram_tensor(
        "xt_full", [N_NODES, HID], dt.float8e4, kind="Internal",
        addr_space="Shared",
    )
    ht_loc = nc.dram_tensor("ht_loc", [PADN, P], dt.bfloat16, kind="Internal")
    ht_full = nc.dram_tensor(
        "ht_full", [N_NODES, P], dt.bfloat16, kind="Internal",
        addr_space="Shared",
    )

    rg = [list(range(NCORES))]
    _qrr = [0]

    def dg_raw(out_ap, in_ap, idxs_ap, num_idxs, elem_size, stride_256,
               queue=None):
        eng = nc.gpsimd
        if queue is None:
            q = _qrr[0]
            _qrr[0] = (q + 1) % 3
        else:
            q = queue
        _in_ap = eng.lower_ap_dma(in_ap, for_custom_bir_dma=True)
        _idxs_ap = eng.lower_ap(idxs_ap)
        _out_ap = eng.lower_ap(out_ap)
        return eng.add_instruction(
            mybir.InstDMAGatherAnt(
                name=nc.get_next_instruction_name(),
                ins=[*_in_ap, _idxs_ap, eng.lower_val_access(eng.to_reg(num_idxs))],
                outs=[_out_ap],
                transpose=False,
                num_idxs=num_idxs,
                elem_size=elem_size,
                stride_bytes_256=stride_256,
                gen_mode=0,
                single_packet=True,
                queue_num=q,
                sbuf_tokens_per_rank=0,
                sbuf_free_dim_per_rank=0,
                sbuf_free_dim_pad_per_rank=0,
                sbuf_byte_offset=0,
            )
        )

    def allgather(src_dram, dst_dram, s):
        a, ln = SLAB_STARTS[s], SLAB_LENS[s]
        if sim_mode:
            for cc in range(NCORES):
                nc.sync.dma_start(
                    dst_dram[8 * a + cc * ln : 8 * a + (cc + 1) * ln, :],
                    src_dram[a : a + ln, :],
                )
        else:
            nc.gpsimd.collective_compute(
                "AllGather",
                A.bypass,
                replica_groups=rg,
                ins=[src_dram[a : a + ln, :]],
                outs=[dst_dram[8 * a : 8 * (a + ln), :]],
            )

    maxGA = max(n for _, n in callsA)
    maxGB = max(n for _, n in callsB)
    maxG = max(
        int(sum(cwA[w] + cwB[w] for w in ws)) for ws in groups
    )


    with tile.TileContext(nc) as tc:
        import contextlib

        ctx = contextlib.ExitStack()
        sb = ctx.enter_context(tc.tile_pool(name="sb", bufs=1))
        sb3 = ctx.enter_context(tc.tile_pool(name="sb3", bufs=3))
        psp = ctx.enter_context(tc.tile_pool(name="psp", bufs=1, space="PSUM"))

        # ---------- resident loads ----------
        src16_sb = sb.tile([P, C * 8], dt.int16)
        nc.sync.dma_start(src16_sb[:], src16_in[:])
        ones_bf_s = sb.tile([1, P], dt.bfloat16)
        nc.sync.dma_start(ones_bf_s[:], ones_bf[:])
        b1row_s = sb.tile([1, HID], dt.bfloat16)
        nc.sync.dma_start(b1row_s[:], b1row[:])
        b2c_s = sb.tile([1, OUTP], dt.bfloat16)
        nc.sync.dma_start(b2c_s[:], b2c[:])
        ident_s = sb.tile([P, P], dt.bfloat16)
        nc.sync.dma_start(ident_s[:], ident_in[:])
        m2T_s = sb.tile([OUTP, PADN], dt.float8e4)
        nc.sync.dma_start(m2T_s[:], m2T_in[:])
        m1_slab = sb.tile([P, NW, HID], dt.float8e4)
        nc.sync.dma_start(
            m1_slab[:], m1_in[0:PADN, :].rearrange("(w p) h -> p w h", p=P)
        )

        W1_s = []
        for k in range(KT1):
            t = sb.tile([P, 2 * HID], dt.bfloat16, name=f"W1_s{k}")
            nc.sync.dma_start(t[:], W1[k * P : (k + 1) * P, :])
            W1_s.append(t)
        W2_s = []
        for k in range(KT2):
            t = sb.tile([P, 2 * OUTP], dt.bfloat16, name=f"W2_s{k}")
            nc.sync.dma_start(t[:], W2[k * P : (k + 1) * P, :])
            W2_s.append(t)

        root1_slab = sb.tile([P, NW * HID], dt.bfloat16)
        root2T_slab = sb.tile([OUTP, PADN], dt.bfloat16)
        hT_slab = []
        for k in range(KT2):
            t = sb.tile([P, PADN], dt.bfloat16, name=f"hT_slab{k}")
            hT_slab.append(t)

        # pre-zeroed fp8 pad buffers for the ht table rows
        htpad = []
        for i in range(2):
            t = sb.tile([P, P], dt.bfloat16, name=f"htpad{i}")
            nc.vector.memset(t[:], 0.0)
            htpad.append(t)

        # ---------- dense1 + chunked AllGather(xt) ----------
        slab_idx = 0
        for m in range(NW):
            ps = psp.tile([P, 2 * HID], dt.float32, space="PSUM", tag="d1", bufs=2)
            xt_k = sb3.tile([P, KT1, P], dt.bfloat16, tag="xTt", bufs=3)
            nc.sync.dma_start(
                xt_k[:],
                xT[:, m * P : (m + 1) * P].rearrange("(k p) n -> p k n", k=KT1),
            )
            for k in range(KT1):
                nc.tensor.matmul(
                    ps[:], lhsT=xt_k[:, k, :], rhs=W1_s[k][:],
                    start=(k == 0), stop=False,
                )
            nc.tensor.matmul(
                ps[:, HID : 2 * HID],
                lhsT=ones_bf_s[:], rhs=b1row_s[:],
                start=False, stop=True,
            )
            xt_t = sb3.tile([P, HID], dt.float8e4, tag="xt_t")
            nc.scalar.copy(xt_t[:], ps[:, 0:HID])
            nc.sync.dma_start(xt_loc[m * P : (m + 1) * P, :], xt_t[:])
            nc.vector.tensor_copy(
                out=root1_slab[:, m * HID : (m + 1) * HID],
                in_=ps[:, HID : 2 * HID],
            )
            if m + 1 == SLAB_WEND[slab_idx]:
                allgather(xt_loc, xt_full, slab_idx)
                slab_idx += 1

        # ---------- edge layer 1 (+ interleaved dense2 + AG(ht)) ----------
        cap = IDXCAP // P
        slab_idx = 0
        for gi, ws in enumerate(groups):
            c0A, nA = callsA[gi]
            c0B, nB = callsB[gi]
            rowsA = sb3.tile([P, maxGA, HID], dt.float8e4, tag="rows1A", bufs=2)
            for o in range(0, nA, cap):
                n_ = min(cap, nA - o)
                dg_raw(
                    rowsA[:, o : o + n_, :], xt_full[0:SPLIT, :],
                    src16_sb[:, (c0A + o) * 8 : (c0A + o + n_) * 8],
                    n_ * P, HID, 1,
                )
            rowsB = sb3.tile([P, maxGB, HID], dt.float8e4, tag="rows1B", bufs=2)
            for o in range(0, nB, cap):
                n_ = min(cap, nB - o)
                dg_raw(
                    rowsB[:, o : o + n_, :], xt_full[SPLIT:, :],
                    src16_sb[:, (c0B + o) * 8 : (c0B + o + n_) * 8],
                    n_ * P, HID, 1, queue=3,
                )
            oh1_t = sb3.tile([P, maxG, P], dt.float8e4, tag="oh1", bufs=2)
            gc0 = c0A
            gcols = nA + nB
            nc.sync.dma_start(
                oh1_t[:, 0:gcols, :], OH1_in[:, gc0 * P : (gc0 + gcols) * P]
            )

            for w in ws:
                acols = [
                    (rowsA, colA[w] - c0A, colA[w] - gc0, int(cwA[w]))
                ]
                bcols = [
                    (rowsB, colB[w] - c0B, colB[w] - gc0, int(cwB[w]))
                ]
                ps_b = psp.tile(
                    [P, HID], dt.float32, space="PSUM", tag="big", bufs=2
                )
                first = True
                for rt, rc0, oc0, ncol in acols + bcols:
                    j = 0
                    while j + 2 <= ncol:
                        nc.tensor.matmul(
                            ps_b[:],
                            lhsT=oh1_t[:, oc0 + j : oc0 + j + 2, :],
                            rhs=rt[:, rc0 + j : rc0 + j + 2, :],
                            start=first, stop=False, perf_mode=DR,
                        )
                        first = False
                        j += 2
                    if j < ncol:
                        nc.tensor.matmul(
                            ps_b[:],
                            lhsT=oh1_t[:, oc0 + j, :],
                            rhs=rt[:, rc0 + j, :],
                            start=first, stop=False,
                        )
                        first = False
                # + root1 (includes b1): identity matmul re-add
                nc.tensor.matmul(
                    ps_b[:],
                    lhsT=ident_s[:],
                    rhs=root1_slab[:, w * HID : (w + 1) * HID],
                    start=False, stop=True,
                )
                t0 = sb3.tile([P, HID], dt.bfloat16, tag="t0", bufs=2)
                nc.vector.tensor_tensor(
                    out=t0[:], in0=ps_b[:], in1=m1_slab[:, w, :], op=A.mult
                )
                h_t = sb3.tile([P, HID], dt.bfloat16, tag="h_t", bufs=2)
                nc.scalar.activation(h_t[:], t0[:], F.Relu, scale=INV_KEEP)

                # dense2 for this window: hT via PE transpose, then matmuls
                tp = psp.tile(
                    [P, 2 * P], dt.bfloat16, space="PSUM", tag="tp", bufs=1
                )
                for k in range(KT2):
                    nc.tensor.transpose(
                        out=tp[:, k * P : (k + 1) * P],
                        in_=h_t[:, k * P : (k + 1) * P],
                        identity=ident_s[:],
                    )
                    nc.scalar.copy(
                        hT_slab[k][:, w * P : (w + 1) * P],
                        tp[:, k * P : (k + 1) * P],
                    )
                psm = psp.tile(
                    [P, OUTP], dt.float32, space="PSUM", tag="pm", bufs=1
                )
                for k in range(KT2):
                    nc.tensor.matmul(
                        psm[:],
                        lhsT=hT_slab[k][:, w * P : (w + 1) * P],
                        rhs=W2_s[k][:, 0:OUTP],
                        start=(k == 0), stop=(k == KT2 - 1),
                    )
                hp = htpad[w % 2]
                nc.scalar.copy(hp[:, 0:OUTP], psm[:])
                nc.sync.dma_start(ht_loc[w * P : (w + 1) * P, :], hp[:])

                psr = psp.tile(
                    [OUTP, P], dt.float32, space="PSUM", tag="pg", bufs=2
                )
                for k in range(KT2):
                    nc.tensor.matmul(
                        psr[:],
                        lhsT=W2_s[k][:, OUTP : 2 * OUTP],
                        rhs=hT_slab[k][:, w * P : (w + 1) * P],
                        start=(k == 0), stop=False,
                    )
                nc.tensor.matmul(
                    psr[:], lhsT=b2c_s[:], rhs=ones_bf_s[:],
                    start=False, stop=True,
                )
                nc.scalar.copy(root2T_slab[:, w * P : (w + 1) * P], psr[:])

                if w + 1 == SLAB_WEND[slab_idx]:
                    allgather(ht_loc, ht_full, slab_idx)
                    slab_idx += 1

        # ---------- edge layer 2 ----------
        for gi, ws in enumerate(groups):
            c0A, nA = callsA[gi]
            c0B, nB = callsB[gi]
            rows2A = sb3.tile([P, maxGA, OUTP], dt.bfloat16, tag="rows2A", bufs=2)
            for o in range(0, nA, cap):
                n_ = min(cap, nA - o)
                dg_raw(
                    rows2A[:, o : o + n_, :], ht_full[0:SPLIT, 0:OUTP],
                    src16_sb[:, (c0A + o) * 8 : (c0A + o + n_) * 8],
                    n_ * P, OUTP, 1,
                )
            rows2B = sb3.tile([P, maxGB, OUTP], dt.bfloat16, tag="rows2B", bufs=2)
            for o in range(0, nB, cap):
                n_ = min(cap, nB - o)
                dg_raw(
                    rows2B[:, o : o + n_, :], ht_full[SPLIT:, 0:OUTP],
                    src16_sb[:, (c0B + o) * 8 : (c0B + o + n_) * 8],
                    n_ * P, OUTP, 1, queue=3,
                )
            oh2_t = sb3.tile([P, maxG, P], dt.float8e4, tag="oh2", bufs=2)
            gc0 = c0A
            gcols = nA + nB
            nc.sync.dma_start(
                oh2_t[:, 0:gcols, :], OH2_in[:, gc0 * P : (gc0 + gcols) * P]
            )

            for w in ws:
                acols = [
                    (rows2A, colA[w] - c0A, colA[w] - gc0, int(cwA[w]))
                ]
                bcols = [
                    (rows2B, colB[w] - c0B, colB[w] - gc0, int(cwB[w]))
                ]
                psg = psp.tile(
                    [OUTP, P], dt.float32, space="PSUM", tag="pg", bufs=2
                )
                first = True
                for rt, rc0, oc0, ncol in acols + bcols:
                    for j in range(ncol):
                        nc.tensor.matmul(
                            psg[:],
                            lhsT=rt[:, rc0 + j, :],
                            rhs=oh2_t[:, oc0 + j, :],
                            start=first, stop=False,
                        )
                        first = False
                # + root2 (includes b2)
                nc.tensor.matmul(
                    psg[:],
                    lhsT=ident_s[0:OUTP, 0:OUTP],
                    rhs=root2T_slab[:, w * P : (w + 1) * P],
                    start=False, stop=True,
                )
                t2 = sb3.tile([OUTP, P], dt.float32, tag="t2", bufs=2)
                nc.vector.tensor_tensor(
                    out=t2[:], in0=psg[:],
                    in1=m2T_s[:, w * P : (w + 1) * P], op=A.mult,
                )
                yt_t = sb3.tile([OUTP, P], dt.float32, tag="yt_t", bufs=2)
                nc.scalar.activation(yt_t[:], t2[:], F.Relu, scale=INV_KEEP)
                nc.sync.dma_start(yT[:, w * P : (w + 1) * P], yt_t[:])
        ctx.close()

    nc.compile()
    return nc


def _build_noop_program(meta=None):
    """Same I/O signature as the real program, near-empty body — used to
    measure PJRT dispatch overhead for wall-clock benchmarking."""
    import concourse.bacc as bacc
    import concourse.mybir as mybir
    import concourse.tile as tile

    dt = mybir.dt
    C = meta["C"] if meta else 848
    nc = bacc.Bacc(
        "TRN2", target_bir_lowering=False, debug=False, num_devices=NCORES,
        dynamic_dma_scratch_size=DDS, num_swdge_queues=4,
    )

    def inp(name, shape, d):
        return nc.dram_tensor(name, shape, d, kind="ExternalInput")

    inp("xT", [IN_DIM, PADN], dt.bfloat16)
    inp("W1", [IN_DIM, 2 * HID], dt.bfloat16)
    inp("W2", [HID, 2 * OUTP], dt.bfloat16)
    inp("b1row", [1, HID], dt.bfloat16)
    inp("b2c", [1, OUTP], dt.bfloat16)
    inp("ones_bf", [1, P], dt.bfloat16)
    inp("ident", [P, P], dt.bfloat16)
    m1 = inp("m1", [PADN, HID], dt.float8e4)
    inp("m2T", [OUTP, PADN], dt.float8e4)
    inp("src16", [P, C * 8], dt.int16)
    inp("OH1", [P, C * P], dt.float8e4)
    inp("OH2", [P, C * P], dt.float8e4)
    yT = nc.dram_tensor("yT", [OUTP, PADN], dt.float32, kind="ExternalOutput")
    with tile.TileContext(nc) as tc:
        with tc.tile_pool(name="sb", bufs=1) as sb:
            t = sb.tile([OUTP, P], dt.float8e4)
            nc.sync.dma_start(t[:], m1[0:OUTP, 0:P])
            t2 = sb.tile([OUTP, P], dt.float32)
            nc.vector.tensor_copy(out=t2[:], in_=t[:])
            nc.sync.dma_start(yT[:, 0:P], t2[:])
    nc.compile()
    return nc


def _stage_inputs(inputs, per_core_arrays):
    x = np.asarray(inputs["x"], np.float32)
    W1m = np.asarray(inputs["W1_msg"], np.float32)
    W1r = np.asarray(inputs["W1_root"], np.float32)
    b1 = np.asarray(inputs["b1"], np.float32)
    W2m = np.asarray(inputs["W2_msg"], np.float32)
    W2r = np.asarray(inputs["W2_root"], np.float32)
    b2 = np.asarray(inputs["b2"], np.float32)
    drop1 = np.asarray(inputs["drop1"], np.float32)
    drop2 = np.asarray(inputs["drop2"], np.float32)

    W1cat = np.concatenate([W1m, W1r], axis=1).astype(bf16)  # [768,512]
    W2cat = np.zeros((HID, 2 * OUTP), np.float32)
    W2cat[:, 0:OUT] = W2m
    W2cat[:, OUTP : OUTP + OUT] = W2r
    W2cat = W2cat.astype(bf16)
    b1row = b1.reshape(1, HID).astype(bf16)
    b2c = np.zeros((1, OUTP), np.float32)
    b2c[0, :OUT] = b2
    b2c = b2c.astype(bf16)
    ones_bf_a = np.ones((1, P), bf16)
    ident_a = np.eye(P, dtype=np.float32).astype(bf16)

    common = {
        "W1": W1cat,
        "W2": W2cat,
        "b1row": b1row,
        "b2c": b2c,
        "ones_bf": ones_bf_a,
        "ident": ident_a,
    }

    in_maps = []
    for c in range(NCORES):
        lo, hi = c * SHARD, (c + 1) * SHARD
        xTp = np.ones((IN_DIM, PADN), np.float32)
        xTp[:, :SHARD] = x[lo:hi].T
        m1p = np.ones((PADN, HID), np.float32)
        m1p[:SHARD] = drop1[lo:hi] >= P_DROP
        m2Tp = np.zeros((OUTP, PADN), np.float32)
        m2Tp[:OUT, :SHARD] = (drop2[lo:hi] >= P_DROP).T
        src16, OH1, OH2 = per_core_arrays[c]
        in_maps.append(
            {
                **common,
                "xT": xTp.astype(bf16),
                "m1": m1p.astype(f8),
                "m2T": m2Tp.astype(f8),
                "src16": src16,
                "OH1": OH1,
                "OH2": OH2,
            }
        )
    return in_maps


def _run(inputs, trace=False, trace_kwargs=None):
    from concourse import bass_utils

    et = np.asarray(inputs["edge_type"]).astype(np.int64)
    ed = np.asarray(inputs["edge_distance"]).astype(np.int64)
    a1 = _edge_alphas(
        et, ed, np.asarray(inputs["te1"], np.float32),
        np.asarray(inputs["de1"], np.float32),
        np.asarray(inputs["g1_w"], np.float32),
        np.asarray(inputs["g1_b"]).reshape(-1)[0],
    )
    a2 = _edge_alphas(
        et, ed, np.asarray(inputs["te2"], np.float32),
        np.asarray(inputs["de2"], np.float32),
        np.asarray(inputs["g2_w"], np.float32),
        np.asarray(inputs["g2_b"]).reshape(-1)[0],
    )
    meta, per_core_arrays = _prep_edges(inputs["edge_index"], a1, a2)
    nc = _build_program(meta)
    in_maps = _stage_inputs(inputs, per_core_arrays)
    res = bass_utils.run_bass_kernel_spmd(
        nc,
        in_maps,
        core_ids=list(range(NCORES)),
        trace=trace,
        **(trace_kwargs or {}),
    )
    parts = []
    for c in range(NCORES):
        yTa = res.results[c]["yT"]
        parts.append(np.ascontiguousarray(yTa[:OUT, :SHARD].T))
    y = np.concatenate(parts, axis=0).astype(np.float32)
    return y, res


def kernel(**inputs) -> np.ndarray:
    y, _ = _run(inputs, trace=False)
    return y


# revision 17
# speedup vs baseline: 1.2086x; 1.0025x over previous
"""Trainium2 Bass kernel for nn_ARGCNNet (2-layer gated relational GCN), v2.

Strategy (8 NeuronCores, graph/data parallel):
  - Nodes sharded by row: core c owns nodes [c*6250, (c+1)*6250).
  - Edges routed to the core owning their dst node, sorted by dst window,
    packed into 128-edge chunks (padding uniform across cores -> one SPMD
    program). Chunks split into A (permuted src < 32768) and B parts because
    dma_gather indices are int16.
  - Per-edge gates alpha1/alpha2 are pure functions of host-known inputs
    (edge_type/edge_distance + small tables) -> computed on HOST.
  - The alpha-scaled one-hot matrices (lhsT of the segment-sum matmuls) are
    HOST-precomputed in fp8e4 and streamed in, killing all on-device one-hot
    DVE work and the per-edge alpha gather.
  - Message path runs in fp8e4: xt = x@W1_msg cast to fp8, AllGather'ed in
    fp8 (half the bytes), per-edge rows gathered as 256B fp8 rows, and the
    segment-sum matmuls run fp8 x fp8 with DoubleRow perf mode (2 chunks per
    matmul, 2x PE rate). Root paths and dense GEMMs stay bf16.
  - AllGathers are chunked into 4 node-slabs and overlapped: AG(xt) slabs
    fire as dense1 finishes each slab; dense2 is interleaved into the edge-1
    loop so AG(ht) slabs fire while edge-1 still runs. Table row ids are
    permuted host-side to match the slab-concatenated AllGather layout.
  - h never touches DRAM: transposed on the PE into an SBUF slab for dense2.
  - Dropout masks are host-precomputed 0/1 fp8; the 1/(1-p) scale is folded
    into the ReLU activations.
"""

import os
import sys

import numpy as np

for _p in ("/opt/trn_rl_repo", "/root/.axon_site/_ro/trn_rl_repo"):
    if os.path.isdir(_p) and _p not in sys.path:
        sys.path.insert(0, _p)

import ml_dtypes

bf16 = ml_dtypes.bfloat16
f8 = ml_dtypes.float8_e4m3  # TRN FP8_EXP4 (matches for |x| <= 240)

N_NODES = 50000
N_EDGES = 800000
IN_DIM = 768
HID = 256
OUT = 9
OUTP = 16
N_TYPES = 50
N_DIST = 128
P_DROP = np.float32(0.4)
INV_KEEP = float(np.float32(1.0) / (np.float32(1.0) - P_DROP))

NCORES = 8
SHARD = N_NODES // NCORES  # 6250
P = 128
NW = (SHARD + P - 1) // P  # 49 windows per core
PADN = NW * P  # 6272
KT1 = IN_DIM // P  # 6
KT2 = HID // P  # 2
SPLIT = 32768  # int16 index limit for dma_gather
GW = 2  # windows per gather group
IDXCAP = 1024  # max indices per dma_gather call
DDS = 65536

# AllGather slabs (core-local row ranges). The slab boundary at local row
# 4096 puts the slab-0/1 table split exactly at 8*4096 = 32768 = SPLIT, so
# A-part gathers (int16 idx < 32768) depend only on slab 0's AllGather.
SLAB_STARTS = [0, 4096]
SLAB_LENS = [4096, 2154]
SLAB_WEND = [32, 49]  # dense window index (exclusive) per slab
NSLAB = 2


def _perm_ids():
    """Global node id -> permuted table row id (slab-concatenated AllGather
    layout: table = [slab0: core0..7 | slab1: core0..7 | ...])."""
    ids = np.arange(N_NODES, dtype=np.int64)
    c = ids // SHARD
    r = ids % SHARD
    s = np.minimum(r // 4096, 1)
    starts = np.asarray(SLAB_STARTS, dtype=np.int64)[s]
    lens = np.asarray(SLAB_LENS, dtype=np.int64)[s]
    return 8 * starts + c * lens + (r - starts)


def _wrap_idx(flat):
    """int16 flat index list -> [128, n/16] wrapped + replicated layout."""
    n = flat.size
    assert n % 16 == 0
    t = np.empty((P, n // 16), np.int16)
    for p in range(16):
        row = flat[p::16]
        for g in range(8):
            t[16 * g + p, :] = row
    return t


def _edge_alphas(et, ed, te, de, gw, gb):
    tg = te.astype(np.float64) @ gw[:100, 0].astype(np.float64)  # [50]
    dg = de.astype(np.float64) @ gw[100:, 0].astype(np.float64)  # [128]
    z = tg[et] + dg[ed] + float(gb)
    return (1.0 / (1.0 + np.exp(-z))).astype(np.float32)


def _prep_edges(edge_index, a1, a2):
    """Route/sort/pack edges; build per-core src16 + fp8 one-hot arrays."""
    src = np.asarray(edge_index[0]).astype(np.int64)
    dst = np.asarray(edge_index[1]).astype(np.int64)
    perm = _perm_ids()
    psrc = perm[src]
    owner = dst // SHARD

    per_core = []
    cntA = np.zeros((NCORES, NW), np.int64)
    cntB = np.zeros((NCORES, NW), np.int64)
    for c in range(NCORES):
        m = owner == c
        dstl = dst[m] - c * SHARD
        ps_ = psrc[m]
        isB = (ps_ >= SPLIT).astype(np.int64)
        wid = dstl >> 7
        key = wid * 2 + isB
        order = np.argsort(key, kind="stable")
        per_core.append(
            (dstl[order], ps_[order], a1[m][order], a2[m][order], isB[order])
        )
        cntA[c] = np.bincount(wid[isB == 0], minlength=NW)
        cntB[c] = np.bincount(wid[isB == 1], minlength=NW)

    cwA = np.maximum(1, (cntA.max(axis=0) + P - 1) // P)  # [NW]
    cwB = np.maximum(1, (cntB.max(axis=0) + P - 1) // P)

    groups = [list(range(g, min(g + GW, NW))) for g in range(0, NW, GW)]
    colA = {}
    colB = {}
    callsA = []  # (col0, ncols) per group
    callsB = []
    cur = 0
    for ws in groups:
        c0 = cur
        for w in ws:
            colA[w] = cur
            cur += int(cwA[w])
        callsA.append((c0, cur - c0))
        c0 = cur
        for w in ws:
            colB[w] = cur
            cur += int(cwB[w])
        callsB.append((c0, cur - c0))
    C = cur

    meta = {
        "cwA": cwA,
        "cwB": cwB,
        "colA": colA,
        "colB": colB,
        "callsA": callsA,
        "callsB": callsB,
        "groups": groups,
        "C": C,
    }

    colA_arr = np.array([colA[w] for w in range(NW)])
    colB_arr = np.array([colB[w] for w in range(NW)])
    per_core_arrays = []
    for c in range(NCORES):
        dstl, ps_, a1c, a2c, isB = per_core[c]
        wid = dstl >> 7
        keys = wid * 2 + isB
        cnt = np.bincount(keys, minlength=2 * NW)
        start = np.concatenate([[0], np.cumsum(cnt)[:-1]])
        rank = np.arange(dstl.size) - start[keys]
        colbase = np.where(isB == 0, colA_arr[wid], colB_arr[wid])
        slot = (colbase + (rank >> 7)) * P + (rank & 127)

        srcrel = np.zeros(C * P, np.int16)
        srcrel[slot] = np.where(isB == 1, ps_ - SPLIT, ps_).astype(np.int16)

        # one-hot (alpha-scaled) lhsT arrays: [slot_p, col, dst_low]
        flat = (slot & 127) * (C * P) + (slot >> 7) * P + (dstl & 127)
        oh = np.zeros(P * C * P, np.float32)
        oh[flat] = a1c
        OH1 = oh.reshape(P, C * P).astype(f8)
        oh[flat] = a2c
        OH2 = oh.reshape(P, C * P).astype(f8)
        per_core_arrays.append((_wrap_idx(srcrel), OH1, OH2))
    return meta, per_core_arrays


def _build_program(meta, sim_mode=False):
    import concourse.bacc as bacc
    import concourse.bass as bass  # noqa: F401
    import concourse.mybir as mybir
    import concourse.tile as tile

    A = mybir.AluOpType
    F = mybir.ActivationFunctionType
    dt = mybir.dt
    DR = mybir.MatmulPerfMode.DoubleRow

    C = meta["C"]
    cwA, cwB = meta["cwA"], meta["cwB"]
    colA, colB = meta["colA"], meta["colB"]
    callsA, callsB = meta["callsA"], meta["callsB"]
    groups = meta["groups"]

    nc = bacc.Bacc(
        "TRN2", target_bir_lowering=False, debug=False,
        num_devices=(1 if sim_mode else NCORES),
        dynamic_dma_scratch_size=DDS,
        num_swdge_queues=4,
    )

    def inp(name, shape, d):
        return nc.dram_tensor(name, shape, d, kind="ExternalInput")

    xT = inp("xT", [IN_DIM, PADN], dt.bfloat16)
    W1 = inp("W1", [IN_DIM, 2 * HID], dt.bfloat16)  # [msg | root]
    W2 = inp("W2", [HID, 2 * OUTP], dt.bfloat16)  # [msg | root] padded
    b1row = inp("b1row", [1, HID], dt.bfloat16)
    b2c = inp("b2c", [1, OUTP], dt.bfloat16)
    ones_bf = inp("ones_bf", [1, P], dt.bfloat16)
    ident_in = inp("ident", [P, P], dt.bfloat16)
    m1_in = inp("m1", [PADN, HID], dt.float8e4)
    m2T_in = inp("m2T", [OUTP, PADN], dt.float8e4)
    src16_in = inp("src16", [P, C * 8], dt.int16)
    OH1_in = inp("OH1", [P, C * P], dt.float8e4)
    OH2_in = inp("OH2", [P, C * P], dt.float8e4)

    yT = nc.dram_tensor("yT", [OUTP, PADN], dt.float32, kind="ExternalOutput")

    xt_loc = nc.dram_tensor("xt_loc", [PADN, HID], dt.float8e4, kind="Internal")
    xt_full = nc.dram_tensor(
        "xt_full", [N_NODES, HID], dt.float8e4, kind="Internal",
        addr_space="Shared",
    )
    ht_loc = nc.dram_tensor("ht_loc", [PADN, P], dt.bfloat16, kind="Internal")
    ht_full = nc.dram_tensor(
        "ht_full", [N_NODES, P], dt.bfloat16, kind="Internal",
        addr_space="Shared",
    )

    rg = [list(range(NCORES))]
    _qrr = [0]

    def dg_raw(out_ap, in_ap, idxs_ap, num_idxs, elem_size, stride_256,
               queue=None):
        eng = nc.gpsimd
        if queue is None:
            q = _qrr[0]
            _qrr[0] = (q + 1) % 3
        else:
            q = queue
        _in_ap = eng.lower_ap_dma(in_ap, for_custom_bir_dma=True)
        _idxs_ap = eng.lower_ap(idxs_ap)
        _out_ap = eng.lower_ap(out_ap)
        return eng.add_instruction(
            mybir.InstDMAGatherAnt(
                name=nc.get_next_instruction_name(),
                ins=[*_in_ap, _idxs_ap, eng.lower_val_access(eng.to_reg(num_idxs))],
                outs=[_out_ap],
                transpose=False,
                num_idxs=num_idxs,
                elem_size=elem_size,
                stride_bytes_256=stride_256,
                gen_mode=0,
                single_packet=True,
                queue_num=q,
                sbuf_tokens_per_rank=0,
                sbuf_free_dim_per_rank=0,
                sbuf_free_dim_pad_per_rank=0,
                sbuf_byte_offset=0,
            )
        )

    def allgather(src_dram, dst_dram, s):
        a, ln = SLAB_STARTS[s], SLAB_LENS[s]
        if sim_mode:
            for cc in range(NCORES):
                nc.sync.dma_start(
                    dst_dram[8 * a + cc * ln : 8 * a + (cc + 1) * ln, :],
                    src_dram[a : a + ln, :],
                )
        else:
            nc.gpsimd.collective_compute(
                "AllGather",
                A.bypass,
                replica_groups=rg,
                ins=[src_dram[a : a + ln, :]],
                outs=[dst_dram[8 * a : 8 * (a + ln), :]],
            )

    maxGA = max(n for _, n in callsA)
    maxGB = max(n for _, n in callsB)
    maxG = max(
        int(sum(cwA[w] + cwB[w] for w in ws)) for ws in groups
    )


    with tile.TileContext(nc) as tc:
        import contextlib

        ctx = contextlib.ExitStack()
        sb = ctx.enter_context(tc.tile_pool(name="sb", bufs=1))
        sb3 = ctx.enter_context(tc.tile_pool(name="sb3", bufs=3))
        psp = ctx.enter_context(tc.tile_pool(name="psp", bufs=1, space="PSUM"))

        # ---------- resident loads ----------
        src16_sb = sb.tile([P, C * 8], dt.int16)
        nc.sync.dma_start(src16_sb[:], src16_in[:])
        ones_bf_s = sb.tile([1, P], dt.bfloat16)
        nc.sync.dma_start(ones_bf_s[:], ones_bf[:])
        b1row_s = sb.tile([1, HID], dt.bfloat16)
        nc.sync.dma_start(b1row_s[:], b1row[:])
        b2c_s = sb.tile([1, OUTP], dt.bfloat16)
        nc.sync.dma_start(b2c_s[:], b2c[:])
        ident_s = sb.tile([P, P], dt.bfloat16)
        nc.sync.dma_start(ident_s[:], ident_in[:])
        m2T_s = sb.tile([OUTP, PADN], dt.float8e4)
        nc.sync.dma_start(m2T_s[:], m2T_in[:])
        m1_slab = sb.tile([P, NW, HID], dt.float8e4)
        nc.sync.dma_start(
            m1_slab[:], m1_in[0:PADN, :].rearrange("(w p) h -> p w h", p=P)
        )

        W1_s = []
        for k in range(KT1):
            t = sb.tile([P, 2 * HID], dt.bfloat16, name=f"W1_s{k}")
            nc.sync.dma_start(t[:], W1[k * P : (k + 1) * P, :])
            W1_s.append(t)
        W2_s = []
        for k in range(KT2):
            t = sb.tile([P, 2 * OUTP], dt.bfloat16, name=f"W2_s{k}")
            nc.sync.dma_start(t[:], W2[k * P : (k + 1) * P, :])
            W2_s.append(t)

        root1_slab = sb.tile([P, NW * HID], dt.bfloat16)
        root2T_slab = sb.tile([OUTP, PADN], dt.bfloat16)
        hT_slab = []
        for k in range(KT2):
            t = sb.tile([P, PADN], dt.bfloat16, name=f"hT_slab{k}")
            hT_slab.append(t)

        # pre-zeroed fp8 pad buffers for the ht table rows
        htpad = []
        for i in range(2):
            t = sb.tile([P, P], dt.bfloat16, name=f"htpad{i}")
            nc.vector.memset(t[:], 0.0)
            htpad.append(t)

        # ---------- dense1 + chunked AllGather(xt) ----------
        slab_idx = 0
        for m in range(NW):
            ps = psp.tile([P, 2 * HID], dt.float32, space="PSUM", tag="d1", bufs=2)
            xt_k = sb3.tile([P, KT1, P], dt.bfloat16, tag="xTt", bufs=3)
            nc.sync.dma_start(
                xt_k[:],
                xT[:, m * P : (m + 1) * P].rearrange("(k p) n -> p k n", k=KT1),
            )
            for k in range(KT1):
                nc.tensor.matmul(
                    ps[:], lhsT=xt_k[:, k, :], rhs=W1_s[k][:],
                    start=(k == 0), stop=False,
                )
            nc.tensor.matmul(
                ps[:, HID : 2 * HID],
                lhsT=ones_bf_s[:], rhs=b1row_s[:],
                start=False, stop=True,
            )
            xt_t = sb3.tile([P, HID], dt.float8e4, tag="xt_t")
            nc.scalar.copy(xt_t[:], ps[:, 0:HID])
            nc.sync.dma_start(xt_loc[m * P : (m + 1) * P, :], xt_t[:])
            nc.vector.tensor_copy(
                out=root1_slab[:, m * HID : (m + 1) * HID],
                in_=ps[:, HID : 2 * HID],
            )
            if m + 1 == SLAB_WEND[slab_idx]:
                allgather(xt_loc, xt_full, slab_idx)
                slab_idx += 1

        # ---------- edge layer 1 (+ interleaved dense2 + AG(ht)) ----------
        cap = IDXCAP // P
        slab_idx = 0
        for gi, ws in enumerate(groups):
            c0A, nA = callsA[gi]
            c0B, nB = callsB[gi]
            rowsA = sb3.tile([P, maxGA, HID], dt.float8e4, tag="rows1A", bufs=2)
            for o in range(0, nA, cap):
                n_ = min(cap, nA - o)
                dg_raw(
                    rowsA[:, o : o + n_, :], xt_full[0:SPLIT, :],
                    src16_sb[:, (c0A + o) * 8 : (c0A + o + n_) * 8],
                    n_ * P, HID, 1,
                )
            rowsB = sb3.tile([P, maxGB, HID], dt.float8e4, tag="rows1B", bufs=2)
            for o in range(0, nB, cap):
                n_ = min(cap, nB - o)
                dg_raw(
                    rowsB[:, o : o + n_, :], xt_full[SPLIT:, :],
                    src16_sb[:, (c0B + o) * 8 : (c0B + o + n_) * 8],
                    n_ * P, HID, 1, queue=3,
                )
            oh1_t = sb3.tile([P, maxG, P], dt.float8e4, tag="oh1", bufs=2)
            gc0 = c0A
            gcols = nA + nB
            nc.sync.dma_start(
                oh1_t[:, 0:gcols, :], OH1_in[:, gc0 * P : (gc0 + gcols) * P]
            )

            for w in ws:
                acols = [
                    (rowsA, colA[w] - c0A, colA[w] - gc0, int(cwA[w]))
                ]
                bcols = [
                    (rowsB, colB[w] - c0B, colB[w] - gc0, int(cwB[w]))
                ]
                ps_b = psp.tile(
                    [P, HID], dt.float32, space="PSUM", tag="big", bufs=2
                )
                first = True
                for rt, rc0, oc0, ncol in acols + bcols:
                    j = 0
                    while j + 2 <= ncol:
                        nc.tensor.matmul(
                            ps_b[:],
                            lhsT=oh1_t[:, oc0 + j : oc0 + j + 2, :],
                            rhs=rt[:, rc0 + j : rc0 + j + 2, :],
                            start=first, stop=False, perf_mode=DR,
                        )
                        first = False
                        j += 2
                    if j < ncol:
                        nc.tensor.matmul(
                            ps_b[:],
                            lhsT=oh1_t[:, oc0 + j, :],
                            rhs=rt[:, rc0 + j, :],
                            start=first, stop=False,
                        )
                        first = False
                # + root1 (includes b1): identity matmul re-add
                nc.tensor.matmul(
                    ps_b[:],
                    lhsT=ident_s[:],
                    rhs=root1_slab[:, w * HID : (w + 1) * HID],
                    start=False, stop=True,
                )
                t0 = sb3.tile([P, HID], dt.bfloat16, tag="t0", bufs=2)
                nc.vector.tensor_tensor(
                    out=t0[:], in0=ps_b[:], in1=m1_slab[:, w, :], op=A.mult
                )
                h_t = sb3.tile([P, HID], dt.bfloat16, tag="h_t", bufs=2)
                nc.scalar.activation(h_t[:], t0[:], F.Relu, scale=INV_KEEP)

                # dense2 for this window: hT via PE transpose, then matmuls
                tp = psp.tile(
                    [P, 2 * P], dt.bfloat16, space="PSUM", tag="tp", bufs=1
                )
                for k in range(KT2):
                    nc.tensor.transpose(
                        out=tp[:, k * P : (k + 1) * P],
                        in_=h_t[:, k * P : (k + 1) * P],
                        identity=ident_s[:],
                    )
                    nc.scalar.copy(
                        hT_slab[k][:, w * P : (w + 1) * P],
                        tp[:, k * P : (k + 1) * P],
                    )
                psm = psp.tile(
                    [P, OUTP], dt.float32, space="PSUM", tag="pm", bufs=1
                )
                for k in range(KT2):
                    nc.tensor.matmul(
                        psm[:],
                        lhsT=hT_slab[k][:, w * P : (w + 1) * P],
                        rhs=W2_s[k][:, 0:OUTP],
                        start=(k == 0), stop=(k == KT2 - 1),
                    )
                hp = htpad[w % 2]
                nc.scalar.copy(hp[:, 0:OUTP], psm[:])
                nc.sync.dma_start(ht_loc[w * P : (w + 1) * P, :], hp[:])

                psr = psp.tile(
                    [OUTP, P], dt.float32, space="PSUM", tag="pg", bufs=2
                )
                for k in range(KT2):
                    nc.tensor.matmul(
                        psr[:],
                        lhsT=W2_s[k][:, OUTP : 2 * OUTP],
                        rhs=hT_slab[k][:, w * P : (w + 1) * P],
                        start=(k == 0), stop=False,
                    )
                nc.tensor.matmul(
                    psr[:], lhsT=b2c_s[:], rhs=ones_bf_s[:],
                    start=False, stop=True,
                )
                nc.scalar.copy(root2T_slab[:, w * P : (w + 1) * P], psr[:])

                if w + 1 == SLAB_WEND[slab_idx]:
                    allgather(ht_loc, ht_full, slab_idx)
                    slab_idx += 1

        # ---------- edge layer 2 ----------
        for gi, ws in enumerate(groups):
            c0A, nA = callsA[gi]
            c0B, nB = callsB[gi]
            rows2A = sb3.tile([P, maxGA, OUTP], dt.bfloat16, tag="rows2A", bufs=2)
            for o in range(0, nA, cap):
                n_ = min(cap, nA - o)
                dg_raw(
                    rows2A[:, o : o + n_, :], ht_full[0:SPLIT, 0:OUTP],
                    src16_sb[:, (c0A + o) * 8 : (c0A + o + n_) * 8],
                    n_ * P, OUTP, 1,
                )
            rows2B = sb3.tile([P, maxGB, OUTP], dt.bfloat16, tag="rows2B", bufs=2)
            for o in range(0, nB, cap):
                n_ = min(cap, nB - o)
                dg_raw(
                    rows2B[:, o : o + n_, :], ht_full[SPLIT:, 0:OUTP],
                    src16_sb[:, (c0B + o) * 8 : (c0B + o + n_) * 8],
                    n_ * P, OUTP, 1, queue=3,
                )
            oh2_t = sb3.tile([P, maxG, P], dt.float8e4, tag="oh2", bufs=2)
            gc0 = c0A
            gcols = nA + nB
            nc.sync.dma_start(
                oh2_t[:, 0:gcols, :], OH2_in[:, gc0 * P : (gc0 + gcols) * P]
            )

            for w in ws:
                acols = [
                    (rows2A, colA[w] - c0A, colA[w] - gc0, int(cwA[w]))
                ]
                bcols = [
                    (rows2B, colB[w] - c0B, colB[w] - gc0, int(cwB[w]))
                ]
                psg = psp.tile(
                    [OUTP, P], dt.float32, space="PSUM", tag="pg", bufs=2
                )
                first = True
                for rt, rc0, oc0, ncol in acols + bcols:
                    for j in range(ncol):
                        nc.tensor.matmul(
                            psg[:],
                            lhsT=rt[:, rc0 + j, :],
                            rhs=oh2_t[:, oc0 + j, :],
                            start=first, stop=False,
                        )
                        first = False
                # + root2 (includes b2)
                nc.tensor.matmul(
                    psg[:],
                    lhsT=ident_s[0:OUTP, 0:OUTP],
                    rhs=root2T_slab[:, w * P : (w + 1) * P],
                    start=False, stop=True,
                )
                t2 = sb3.tile([OUTP, P], dt.float32, tag="t2", bufs=2)
                nc.vector.tensor_tensor(
                    out=t2[:], in0=psg[:],
                    in1=m2T_s[:, w * P : (w + 1) * P], op=A.mult,
                )
                yt_t = sb3.tile([OUTP, P], dt.float32, tag="yt_t", bufs=2)
                nc.scalar.activation(yt_t[:], t2[:], F.Relu, scale=INV_KEEP)
                nc.sync.dma_start(yT[:, w * P : (w + 1) * P], yt_t[:])
        ctx.close()

    nc.compile()
    return nc


def _build_noop_program(meta=None):
    """Same I/O signature as the real program, near-empty body — used to
    measure PJRT dispatch overhead for wall-clock benchmarking."""
    import concourse.bacc as bacc
    import concourse.mybir as mybir
    import concourse.tile as tile

    dt = mybir.dt
    C = meta["C"] if meta else 848
    nc = bacc.Bacc(
        "TRN2", target_bir_lowering=False, debug=False, num_devices=NCORES,
        dynamic_dma_scratch_size=DDS, num_swdge_queues=4,
    )

    def inp(name, shape, d):
        return nc.dram_tensor(name, shape, d, kind="ExternalInput")

    inp("xT", [IN_DIM, PADN], dt.bfloat16)
    inp("W1", [IN_DIM, 2 * HID], dt.bfloat16)
    inp("W2", [HID, 2 * OUTP], dt.bfloat16)
    inp("b1row", [1, HID], dt.bfloat16)
    inp("b2c", [1, OUTP], dt.bfloat16)
    inp("ones_bf", [1, P], dt.bfloat16)
    inp("ident", [P, P], dt.bfloat16)
    m1 = inp("m1", [PADN, HID], dt.float8e4)
    inp("m2T", [OUTP, PADN], dt.float8e4)
    inp("src16", [P, C * 8], dt.int16)
    inp("OH1", [P, C * P], dt.float8e4)
    inp("OH2", [P, C * P], dt.float8e4)
    yT = nc.dram_tensor("yT", [OUTP, PADN], dt.float32, kind="ExternalOutput")
    with tile.TileContext(nc) as tc:
        with tc.tile_pool(name="sb", bufs=1) as sb:
            t = sb.tile([OUTP, P], dt.float8e4)
            nc.sync.dma_start(t[:], m1[0:OUTP, 0:P])
            t2 = sb.tile([OUTP, P], dt.float32)
            nc.vector.tensor_copy(out=t2[:], in_=t[:])
            nc.sync.dma_start(yT[:, 0:P], t2[:])
    nc.compile()
    return nc


def _stage_inputs(inputs, per_core_arrays):
    x = np.asarray(inputs["x"], np.float32)
    W1m = np.asarray(inputs["W1_msg"], np.float32)
    W1r = np.asarray(inputs["W1_root"], np.float32)
    b1 = np.asarray(inputs["b1"], np.float32)
    W2m = np.asarray(inputs["W2_msg"], np.float32)
    W2r = np.asarray(inputs["W2_root"], np.float32)
    b2 = np.asarray(inputs["b2"], np.float32)
    drop1 = np.asarray(inputs["drop1"], np.float32)
    drop2 = np.asarray(inputs["drop2"], np.float32)

    W1cat = np.concatenate([W1m, W1r], axis=1).astype(bf16)  # [768,512]
    W2cat = np.zeros((HID, 2 * OUTP), np.float32)
    W2cat[:, 0:OUT] = W2m
    W2cat[:, OUTP : OUTP + OUT] = W2r
    W2cat = W2cat.astype(bf16)
    b1row = b1.reshape(1, HID).astype(bf16)
    b2c = np.zeros((1, OUTP), np.float32)
    b2c[0, :OUT] = b2
    b2c = b2c.astype(bf16)
    ones_bf_a = np.ones((1, P), bf16)
    ident_a = np.eye(P, dtype=np.float32).astype(bf16)

    common = {
        "W1": W1cat,
        "W2": W2cat,
        "b1row": b1row,
        "b2c": b2c,
        "ones_bf": ones_bf_a,
        "ident": ident_a,
    }

    in_maps = []
    for c in range(NCORES):
        lo, hi = c * SHARD, (c + 1) * SHARD
        xTp = np.ones((IN_DIM, PADN), np.float32)
        xTp[:, :SHARD] = x[lo:hi].T
        m1p = np.ones((PADN, HID), np.float32)
        m1p[:SHARD] = drop1[lo:hi] >= P_DROP
        m2Tp = np.zeros((OUTP, PADN), np.float32)
        m2Tp[:OUT, :SHARD] = (drop2[lo:hi] >= P_DROP).T
        src16, OH1, OH2 = per_core_arrays[c]
        in_maps.append(
            {
                **common,
                "xT": xTp.astype(bf16),
                "m1": m1p.astype(f8),
                "m2T": m2Tp.astype(f8),
                "src16": src16,
                "OH1": OH1,
                "OH2": OH2,
            }
        )
    return in_maps


def _run(inputs, trace=False, trace_kwargs=None):
    from concourse import bass_utils

    et = np.asarray(inputs["edge_type"]).astype(np.int64)
    ed = np.asarray(inputs["edge_distance"]).astype(np.int64)
    a1 = _edge_alphas(
        et, ed, np.asarray(inputs["te1"], np.float32),
        np.asarray(inputs["de1"], np.float32),
        np.asarray(inputs["g1_w"], np.float32),
        np.asarray(inputs["g1_b"]).reshape(-1)[0],
    )
    a2 = _edge_alphas(
        et, ed, np.asarray(inputs["te2"], np.float32),
        np.asarray(inputs["de2"], np.float32),
        np.asarray(inputs["g2_w"], np.float32),
        np.asarray(inputs["g2_b"]).reshape(-1)[0],
    )
    meta, per_core_arrays = _prep_edges(inputs["edge_index"], a1, a2)
    nc = _build_program(meta)
    in_maps = _stage_inputs(inputs, per_core_arrays)
    res = bass_utils.run_bass_kernel_spmd(
        nc,
        in_maps,
        core_ids=list(range(NCORES)),
        trace=trace,
        **(trace_kwargs or {}),
    )
    parts = []
    for c in range(NCORES):
        yTa = res.results[c]["yT"]
        parts.append(np.ascontiguousarray(yTa[:OUT, :SHARD].T))
    y = np.concatenate(parts, axis=0).astype(np.float32)
    return y, res


def kernel(**inputs) -> np.ndarray:
    y, _ = _run(inputs, trace=False)
    return y
